# revision 2
# baseline (speedup 1.0000x reference)
"""AttentionHead kernel for Trainium2, 8 NeuronCores, data-parallel over batch.

Problem (fixed shapes):
    input_tensor [8, 2048, 1024] f32, attention_mask [8, 2048] int64 (0/1),
    Wq/Wk/Wv [1024, 128] f32, bq/bk/bv [128] f32.
    out = softmax(mask(Q @ K^T / sqrt(2048))) @ V    -> [8, 2048, 128] f32

Sharding: one batch element per core (B == n_cores == 8). No collectives.

Per-core device kernel (bf16 compute, f32 accumulation):
  - Host pre-transposes X -> XT [1024, 2048] and folds 1/sqrt(S) into Wq/bq.
  - QT/KT [128(e), 2048(tok)] = Wq^T/Wk^T @ XT (PE, K=1024 accumulated).
  - V_ext [2048(key), 129] = XT^T @ [Wv | 0] + [bv | 1]; then each key row is
    multiplied by mask(key) in {0,1}. Masked keys therefore contribute 0 to
    both the attention numerator and the denominator (col 128 of V_ext).
  - S^T tiles [128(key), 512(q)] = KT_chunk^T @ QT_block (PE), exp on ScalarE
    (no max-subtraction needed: |scores| <= ~2 by construction).
  - O_ext [128(q), 129] accumulates exp(S)^T @ V_ext over the 16 key chunks;
    col 128 is the softmax denominator; DVE reciprocal + per-partition scale.
"""

import sys
import types

for _p in ("/opt/trn_rl_repo", "/root/.axon_site/_ro/trn_rl_repo"):
    if _p not in sys.path:
        sys.path.append(_p)

import numpy as np
import ml_dtypes

B, S, DIN, DOUT = 8, 2048, 1024, 128
NCHUNK = DIN // 128          # 8 contraction chunks
NKEY = S // 128              # 16 key chunks
QBLK = 512                   # query block (free dim of S^T matmuls)
NQB = S // QBLK              # 4 query blocks
STG = 2                      # key chunks per exp group ([128, STG*512] psum)
DV = DOUT + 1                # V extended with the ones column

BF16 = ml_dtypes.bfloat16


def _build():
    import concourse.bass as bass
    import concourse.tile as tile
    from concourse import bacc, mybir

    f32 = mybir.dt.float32
    bf16 = mybir.dt.bfloat16
    Exp = mybir.ActivationFunctionType.Exp

    nc = bacc.Bacc("TRN2", target_bir_lowering=False, debug=False, num_devices=B)

    xt_d = nc.dram_tensor("xt", [DIN, S], bf16, kind="ExternalInput")
    wq_d = nc.dram_tensor("wq", [DIN, DOUT], bf16, kind="ExternalInput")
    wk_d = nc.dram_tensor("wk", [DIN, DOUT], bf16, kind="ExternalInput")
    wv_d = nc.dram_tensor("wv", [DIN, DV], bf16, kind="ExternalInput")
    bq_d = nc.dram_tensor("bq", [1, DOUT], bf16, kind="ExternalInput")
    bk_d = nc.dram_tensor("bk", [1, DOUT], bf16, kind="ExternalInput")
    bv_d = nc.dram_tensor("bv", [1, DV], bf16, kind="ExternalInput")
    m01_d = nc.dram_tensor("m01", [128, NKEY], f32, kind="ExternalInput")
    out_d = nc.dram_tensor("out", [S, DOUT], f32, kind="ExternalOutput")

    with tile.TileContext(nc) as tc:
        with (
            tc.tile_pool(name="persist", bufs=1) as pp,
            tc.tile_pool(name="evict", bufs=2 * NKEY // STG) as ep,
            tc.tile_pool(name="small", bufs=4) as sp,
            tc.tile_pool(name="outp", bufs=3) as op,
        ):
            xt = pp.tile([128, NCHUNK * S], bf16, tag="xt")
            wq = pp.tile([128, NCHUNK * DOUT], bf16, tag="wq")
            wk = pp.tile([128, NCHUNK * DOUT], bf16, tag="wk")
            wv = pp.tile([128, NCHUNK * DV], bf16, tag="wv")
            bq = pp.tile([1, DOUT], bf16, tag="bq")
            bk = pp.tile([1, DOUT], bf16, tag="bk")
            bv = pp.tile([1, DV], bf16, tag="bv")
            m01 = pp.tile([128, NKEY], f32, tag="m01")
            ones = pp.tile([1, QBLK], bf16, tag="ones")
            qt = pp.tile([128, S], bf16, tag="qt")
            kt = pp.tile([128, S], bf16, tag="kt")
            vx = pp.tile([128, NKEY * DV], bf16, tag="vx")

            wq3 = wq_d.ap().rearrange("(c p) e -> p c e", p=128)
            wk3 = wk_d.ap().rearrange("(c p) e -> p c e", p=128)
            wv3 = wv_d.ap().rearrange("(c p) e -> p c e", p=128)
            nc.sync.dma_start(wq[:].rearrange("p (c e) -> p c e", c=NCHUNK), wq3)
            nc.sync.dma_start(wk[:].rearrange("p (c e) -> p c e", c=NCHUNK), wk3)
            nc.sync.dma_start(wv[:].rearrange("p (c e) -> p c e", c=NCHUNK), wv3)
            nc.sync.dma_start(bq[:], bq_d.ap())
            nc.sync.dma_start(bk[:], bk_d.ap())
            nc.sync.dma_start(bv[:], bv_d.ap())
            nc.sync.dma_start(m01[:], m01_d.ap())
            nc.vector.memset(ones[:], 1.0)

            xt3 = xt_d.ap().rearrange("(c p) m -> p c m", p=128)
            for c in range(NCHUNK):
                nc.sync.dma_start(xt[:, c * S:(c + 1) * S], xt3[:, c, :])

            # ---- Phase A: QT / KT projections (c-outer for DMA overlap) ----
            with tc.tile_pool(name="ps_a", bufs=2 * NQB, space="PSUM") as ps_a:
                pq = [ps_a.tile([128, QBLK], f32, tag="a", name=f"pq{t}") for t in range(NQB)]
                pk = [ps_a.tile([128, QBLK], f32, tag="a", name=f"pk{t}") for t in range(NQB)]
                for c in range(NCHUNK):
                    for t in range(NQB):
                        nc.tensor.matmul(
                            pq[t][:],
                            wq[:, c * DOUT:(c + 1) * DOUT],
                            xt[:, c * S + t * QBLK: c * S + (t + 1) * QBLK],
                            start=(c == 0), stop=False,
                        )
                    for t in range(NQB):
                        nc.tensor.matmul(
                            pk[t][:],
                            wk[:, c * DOUT:(c + 1) * DOUT],
                            xt[:, c * S + t * QBLK: c * S + (t + 1) * QBLK],
                            start=(c == 0), stop=False,
                        )
                for t in range(NQB):
                    nc.tensor.matmul(pq[t][:], bq[:], ones[:], start=False, stop=True)
                    nc.tensor.matmul(pk[t][:], bk[:], ones[:], start=False, stop=True)
                for t in range(NQB):
                    nc.vector.tensor_copy(qt[:, t * QBLK:(t + 1) * QBLK], pq[t][:])
                    nc.vector.tensor_copy(kt[:, t * QBLK:(t + 1) * QBLK], pk[t][:])

            # ---- Phase B: V_ext (natural layout, keys on partitions) ----
            with tc.tile_pool(name="ps_b", bufs=4, space="PSUM") as ps_b:
                for k in range(NKEY):
                    pv = ps_b.tile([128, DV], f32, tag="v")
                    for c in range(NCHUNK):
                        nc.tensor.matmul(
                            pv[:],
                            xt[:, c * S + k * 128: c * S + (k + 1) * 128],
                            wv[:, c * DV:(c + 1) * DV],
                            start=(c == 0), stop=False,
                        )
                    nc.tensor.matmul(
                        pv[:], ones[:, :128], bv[:], start=False, stop=True
                    )
                    # mask fold: zero rows of V_ext for masked keys (incl. ones col)
                    nc.vector.tensor_scalar_mul(
                        vx[:, k * DV:(k + 1) * DV], pv[:], m01[:, k:k + 1]
                    )

            # ---- Phase C: attention ----
            with (
                tc.tile_pool(name="ps_st", bufs=2, space="PSUM") as ps_st,
                tc.tile_pool(name="ps_o", bufs=2, space="PSUM") as ps_o,
            ):
                ngrp = NKEY // STG
                for t in range(NQB):
                    egs = []
                    for g in range(ngrp):
                        pst = ps_st.tile([128, STG * QBLK], f32, tag="st")
                        for jj in range(STG):
                            j = g * STG + jj
                            nc.tensor.matmul(
                                pst[:, jj * QBLK:(jj + 1) * QBLK],
                                kt[:, j * 128:(j + 1) * 128],
                                qt[:, t * QBLK:(t + 1) * QBLK],
                                start=True, stop=True,
                            )
                        eg = ep.tile([128, STG * QBLK], bf16, tag="e")
                        nc.scalar.activation(eg[:], pst[:], Exp)
                        egs.append(eg)
                    for s in range(NQB):
                        po = ps_o.tile([128, DV], f32, tag="o")
                        for j in range(NKEY):
                            g, jj = j // STG, j % STG
                            nc.tensor.matmul(
                                po[:],
                                egs[g][:, jj * QBLK + s * 128: jj * QBLK + (s + 1) * 128],
                                vx[:, j * DV:(j + 1) * DV],
                                start=(j == 0), stop=(j == NKEY - 1),
                            )
                        rec = sp.tile([128, 1], f32, tag="rec")
                        nc.vector.reciprocal(rec[:], po[:, DOUT:DV])
                        osb = op.tile([128, DOUT], f32, tag="osb")
                        nc.vector.tensor_scalar_mul(osb[:], po[:, :DOUT], rec[:])
                        r0 = t * QBLK + s * 128
                        nc.sync.dma_start(out_d.ap()[r0:r0 + 128, :], osb[:])

    nc.compile()
    return nc


_NC = None


def _get_nc():
    global _NC
    if _NC is None:
        _NC = _build()
    return _NC


def _prep_in_maps(input_tensor, attention_mask, Wq, bq, Wk, bk, Wv, bv):
    scale = np.float32(1.0 / np.sqrt(np.float32(S)))
    wq_h = (np.asarray(Wq, np.float32) * scale).astype(BF16)
    wk_h = np.asarray(Wk, np.float32).astype(BF16)
    wv_h = np.zeros((DIN, DV), dtype=BF16)
    wv_h[:, :DOUT] = np.asarray(Wv, np.float32).astype(BF16)
    bq_h = (np.asarray(bq, np.float32) * scale).astype(BF16).reshape(1, DOUT)
    bk_h = np.asarray(bk, np.float32).astype(BF16).reshape(1, DOUT)
    bv_h = np.zeros((1, DV), dtype=BF16)
    bv_h[0, :DOUT] = np.asarray(bv, np.float32).astype(BF16)
    bv_h[0, DOUT] = BF16(1.0)

    x = np.asarray(input_tensor, np.float32)
    m = np.asarray(attention_mask)
    in_maps = []
    for b in range(B):
        xt_h = np.ascontiguousarray(x[b].T).astype(BF16)            # [DIN, S]
        m01_h = np.ascontiguousarray(
            m[b].astype(np.float32).reshape(NKEY, 128).T)           # [128, NKEY]
        in_maps.append({
            "xt": xt_h, "wq": wq_h, "wk": wk_h, "wv": wv_h,
            "bq": bq_h, "bk": bk_h, "bv": bv_h, "m01": m01_h,
        })
    return in_maps


def run(in_maps, trace=False, **kwargs):
    from concourse.bass_utils import run_bass_kernel_spmd

    nc = _get_nc()
    return run_bass_kernel_spmd(
        nc, in_maps, core_ids=list(range(B)), trace=trace, **kwargs
    )


def kernel(input_tensor, attention_mask, Wq, bq, Wk, bk, Wv, bv):
    in_maps = _prep_in_maps(
        input_tensor, attention_mask, Wq, bq, Wk, bk, Wv, bv)
    res = run(in_maps, trace=False)
    out = np.stack([res.results[b]["out"] for b in range(B)])
    return np.ascontiguousarray(out.astype(np.float32))
